# revision 10
# baseline (speedup 1.0000x reference)
"""Tensor-parallel GQA attention forward for Trainium2, 8 NeuronCores.

Problem: nn_Attention (B=2, T=2048, D=4096, 32 q heads, 8 kv heads, hd=128).

Sharding (tensor-parallel over heads):
  - core c owns q heads 4c..4c+3 (512 features) and kv head c (128 features)
  - wq/wk/wv column-sharded, wo row-sharded; x replicated (pre-transposed on
    host to x^T [D, B*T] so projections need no on-device transpose)
  - each core returns its partial y @ wo_rows contribution; the host sums the
    8 partials (the unshard step for row-sharded wo).

Matmuls run in bf16 with fp32 PSUM accumulation; softmax sums/reciprocal
stay fp32 (denominators via column-group-tiled ones-matmuls).

Device dataflow per core:
  P1: q^T/k^T/v^T = W^T x^T (PSUM accum over 32 d-chunks; weights preloaded
      with a few large DMAs ordered wk,wv,wq so the PE is never starved),
      RoPE fused on q^T/k^T via host-permuted even/odd feature order,
      v transposed to [token, d] tiles via bf16 PE transpose.
  P2: per (batch, head, 512-query block): scores^T = k^T.T @ q^T in pairs of
      key chunks -> one Exp per pair (ScalarE) -> causal band mask on the 4
      diagonal chunks -> y~^T = v.T @ attn^T (PSUM accum); denominators via a
      bf16 DVE pair-sum + one ones-matmul per pair -> y = y~^T * reciprocal.
  P3: out_partial = y^T.T @ wo_rows (PSUM accum over 4 feature chunks).
"""

import sys
import types

import numpy as np
import ml_dtypes

BF = ml_dtypes.bfloat16

B = 2
T = 2048
D = 4096
BT = B * T
NH = 32
NKV = 8
HD = 128
N_CORES = 8
QH = NH // N_CORES          # 4 q heads per core
QF = QH * HD                # 512 q features per core
KF = HD                     # 128 kv features per core
TCH = 256                   # phase-1 token chunk
NTC = BT // TCH             # 16 chunks
DC = D // 128               # 32 contraction chunks
QB = 512                    # phase-2 query block
NQB = T // QB               # 4 blocks per (batch, head)
SCALE = 1.0 / float(np.sqrt(HD))


def _install_ntff_hook_shim():
    """antenv.axon_hooks is absent in this image; synthesize it so
    run_bass_kernel_spmd(trace=True) can profile via libaxon_pjrt.so."""
    try:
        from antenv import axon_hooks  # noqa: F401
        return
    except ImportError:
        pass
    try:
        from trn_agent_boot.trn_boot import _ntff_profile_via_ctypes
        hook = _ntff_profile_via_ctypes("/opt/axon/libaxon_pjrt.so")
    except Exception:
        hook = None
    mod = types.ModuleType("antenv.axon_hooks")
    mod._hook = hook
    mod.get_axon_ntff_profile_hook = lambda: mod._hook

    def _set(h):
        mod._hook = h

    mod.set_axon_ntff_profile_hook = _set
    sys.modules["antenv.axon_hooks"] = mod


_install_ntff_hook_shim()

import concourse.bass as bass  # noqa: E402,F401
import concourse.bacc as bacc  # noqa: E402
import concourse.tile as tile  # noqa: E402
import concourse.mybir as mybir  # noqa: E402
from concourse import bass_utils  # noqa: E402
from concourse.masks import make_identity  # noqa: E402

F32 = mybir.dt.float32
F32R = mybir.dt.float32r
BF16 = mybir.dt.bfloat16
EXP = mybir.ActivationFunctionType.Exp

_NC_CACHE = []


def build():
    nc = bacc.Bacc("TRN2", target_bir_lowering=False, debug=False,
                   num_devices=N_CORES)

    xT = nc.dram_tensor("xT", [128, NTC, DC, TCH], BF16, kind="ExternalInput").ap()
    wq = nc.dram_tensor("wq", [128, DC, QF], BF16, kind="ExternalInput").ap()
    wk = nc.dram_tensor("wk", [128, DC, KF], BF16, kind="ExternalInput").ap()
    wv = nc.dram_tensor("wv", [128, DC, KF], BF16, kind="ExternalInput").ap()
    wo = nc.dram_tensor("wo", [128, QH, D], BF16, kind="ExternalInput").ap()
    cc = nc.dram_tensor("cc", [128, BT], F32, kind="ExternalInput").ap()
    ss = nc.dram_tensor("ss", [128, BT], F32, kind="ExternalInput").ap()
    out = nc.dram_tensor("out", [BT, D], BF16, kind="ExternalOutput").ap()

    with tile.TileContext(nc) as tc:
        _build_body(nc, tc, xT, wq, wk, wv, wo, cc, ss, out)
    nc.compile()
    return nc


def _build_body(nc, tc, xT, wq, wk, wv, wo, cc, ss, out):
    dram = tc.alloc_tile_pool(name="dram", bufs=1, space="DRAM")
    const = tc.alloc_tile_pool(name="const", bufs=1)
    # right-side pools live across the whole kernel (no aliasing with the
    # released phase-1 pools -> their DMAs never gate on phase-1 matmuls)
    wopool = tc.alloc_tile_pool(name="wo", bufs=1, side="right")
    kvpool_b0 = tc.alloc_tile_pool(name="kv0", bufs=1, side="right")
    qpool = tc.alloc_tile_pool(name="q", bufs=3, side="right")
    # PSUM: phase 1 uses ps_proj (4 banks) + ps_t (transposes); phase 2/3
    # use ps4 (2x2 banks) + ps_one ring (4 single banks).
    ps_proj = tc.alloc_tile_pool(name="ps_proj", bufs=4, space="PSUM")
    ps_t = tc.alloc_tile_pool(name="ps_t", bufs=2, space="PSUM")
    # phase-1 pools (released after phase 1 is emitted)
    wpool = tc.alloc_tile_pool(name="weights", bufs=1)
    xpool = tc.alloc_tile_pool(name="xstream", bufs=3)
    cspool = tc.alloc_tile_pool(name="cs", bufs=3)
    rpool = tc.alloc_tile_pool(name="rope", bufs=4)

    # ---- DRAM scratch (per-batch split for coarse phase overlap) ----
    qT_s = [dram.tile([QH, 128, T], BF16, tag=f"qTs{b}", name=f"qTs{b}") for b in range(B)]
    kT_s = [dram.tile([128, T], BF16, tag=f"kTs{b}", name=f"kTs{b}") for b in range(B)]
    v_s = [dram.tile([128, T // 128, 128], BF16, tag=f"vs{b}", name=f"vs{b}") for b in range(B)]
    yT_s = [dram.tile([QH, 128, T], BF16, tag=f"yTs{b}", name=f"yTs{b}") for b in range(B)]

    wo_sb = wopool.tile([128, QH, D], BF16)

    # ---- constants ----
    ident = const.tile([128, 128], F32)
    make_identity(nc, ident[:])
    ident_bf = const.tile([128, 128], BF16)
    nc.vector.tensor_copy(ident_bf[:], ident[:])
    onesPP = const.tile([128, 128], BF16)
    nc.vector.memset(onesPP[:], 1.0)
    # BIG[p, v] = 1.0 iff v - 384 >= p ; mask(delta) = BIG[:, 384-delta :][:QB]
    BIGf = const.tile([128, 896], F32)
    nc.gpsimd.memset(BIGf[:], 1.0)
    nc.gpsimd.affine_select(
        out=BIGf[:], in_=BIGf[:], compare_op=mybir.AluOpType.is_ge,
        fill=0.0, base=-384, channel_multiplier=-1, pattern=[[1, 896]],
    )
    BIG = const.tile([128, 896], BF16)
    nc.vector.tensor_copy(BIG[:], BIGf[:])

    # ---- phase 1: projections + RoPE ----
    # DMA order on the sync queue: x chunk 0, cos/sin 0, then weights as a
    # few large transfers (wk, wv first: the k/v projections of chunk 0 run
    # while wq streams). The x ring (bufs=3) gives a 2-chunk runway.
    def fetch_x(t):
        xts = []
        for sx in range(4):
            xst = xpool.tile([128, DC // 4, TCH], BF16, tag=f"xt{sx}",
                             name=f"xt{sx}")
            nc.sync.dma_start(
                xst[:], xT[:, t, sx * (DC // 4):(sx + 1) * (DC // 4), :])
            xts.append(xst)
        return xts

    def fetch_cs(t):
        tsl = slice(t * TCH, (t + 1) * TCH)
        cc_t = cspool.tile([128, TCH], F32, tag="cc")
        nc.scalar.dma_start(cc_t[:], cc[:, tsl])
        ss_t = cspool.tile([128, TCH], F32, tag="ss")
        nc.scalar.dma_start(ss_t[:], ss[:, tsl])
        return cc_t, ss_t

    cur_x = fetch_x(0)
    cur_cs = fetch_cs(0)

    wk_sb = wpool.tile([128, DC, KF], BF16, name="wk_sb")
    for piece in range(4):
        nc.scalar.dma_start(wk_sb[:, piece * 8:(piece + 1) * 8, :],
                            wk[:, piece * 8:(piece + 1) * 8, :])
    wv_sb = wpool.tile([128, DC, KF], BF16, name="wv_sb")
    for half in range(2):
        nc.scalar.dma_start(wv_sb[:, half * 16:(half + 1) * 16, :],
                            wv[:, half * 16:(half + 1) * 16, :])
    wq_sb = wpool.tile([128, DC, QF], BF16, name="wq_sb")
    for piece in range(8):
        nc.sync.dma_start(wq_sb[:, piece * 4:(piece + 1) * 4, :],
                          wq[:, piece * 4:(piece + 1) * 4, :])

    def rope_evict(ps, cc_t, ss_t, dst_ap):
        """psum [128, TCH] -> RoPE (fp32) -> bf16 -> DMA to dst_ap."""
        raw = rpool.tile([128, TCH], F32, tag="rraw")
        nc.any.tensor_copy(raw[:], ps[:])
        swp = rpool.tile([128, TCH], F32, tag="rswp")
        nc.vector.tensor_copy(swp[0:64, :], raw[64:128, :])
        nc.vector.tensor_copy(swp[64:128, :], raw[0:64, :])
        t1 = rpool.tile([128, TCH], BF16, tag="rt1")
        nc.vector.tensor_mul(out=swp[:], in0=swp[:], in1=ss_t[:])
        nc.vector.tensor_mul(out=t1[:], in0=raw[:], in1=cc_t[:])
        nc.vector.tensor_add(out=t1[:], in0=t1[:], in1=swp[:])
        nc.sync.dma_start(dst_ap, t1[:])

    kv_pre = ([], [])
    for t in range(NTC):
        b, tloc = divmod(t * TCH, T)
        lsl = slice(tloc, tloc + TCH)
        xts = cur_x
        cc_t, ss_t = cur_cs
        if t + 1 < NTC:
            cur_x = fetch_x(t + 1)
            cur_cs = fetch_cs(t + 1)

        ps = ps_proj.tile([128, 512], F32, tag="big", name="ps")[:, :TCH]
        for dc in range(DC):
            nc.tensor.matmul(ps[:], wk_sb[:, dc, :], xts[dc // 8][:, dc % 8, :],
                             start=(dc == 0), stop=(dc == DC - 1))
        rope_evict(ps, cc_t, ss_t, kT_s[b][:, lsl])

        ps = ps_proj.tile([128, 512], F32, tag="big", name="ps")[:, :TCH]
        for dc in range(DC):
            nc.tensor.matmul(ps[:], wv_sb[:, dc, :], xts[dc // 8][:, dc % 8, :],
                             start=(dc == 0), stop=(dc == DC - 1))
        vraw = rpool.tile([128, TCH], BF16, tag="vraw")
        nc.any.tensor_copy(vraw[:], ps[:])
        for j in range(TCH // 128):
            pst = ps_t.tile([128, 128], BF16, tag="pst", name="pst")
            nc.tensor.transpose(pst[:], vraw[:, j * 128:(j + 1) * 128],
                                ident_bf[:])
            vt = rpool.tile([128, 128], BF16, tag="vt")
            nc.any.tensor_copy(vt[:], pst[:])
            g = (tloc // 128) + j
            nc.sync.dma_start(v_s[b][:, g, :], vt[:])

        for fc in range(QH):
            ps = ps_proj.tile([128, 512], F32, tag="big", name="ps")[:, :TCH]
            for dc in range(DC):
                nc.tensor.matmul(
                    ps[:], wq_sb[:, dc, fc * 128:(fc + 1) * 128],
                    xts[dc // 8][:, dc % 8, :],
                    start=(dc == 0), stop=(dc == DC - 1))
            rope_evict(ps, cc_t, ss_t, qT_s[b][fc][:, lsl])

        if t == 2:
            # wo preload on the (otherwise idle) gpsimd SWDGE queue; delayed
            # past startup so it doesn't steal DMA bandwidth from x0/weights
            for fc in range(QH):
                nc.gpsimd.dma_start(wo_sb[:, fc, :], wo[:, fc, :])
        if t == 8:
            # b0 k/v prefetch on the scalar DMA queue (bypasses the x-stream
            # FIFO); data deps (chunk 0-7 scratch writes) already satisfied.
            for kc in range(T // 128):
                ktile = kvpool_b0.tile([128, 128], BF16, tag=f"k0t{kc}",
                                       name=f"k0t{kc}")
                nc.scalar.dma_start(ktile[:], kT_s[0][:, kc * 128:(kc + 1) * 128])
                kv_pre[0].append(ktile)
                vtile = kvpool_b0.tile([128, 128], BF16, tag=f"v0t{kc}",
                                       name=f"v0t{kc}")
                nc.scalar.dma_start(vtile[:], v_s[0][:, kc, :])
                kv_pre[1].append(vtile)

    rpool.release()
    cspool.release()
    xpool.release()
    wpool.release()
    ps_t.release()
    ps_proj.release()

    kvpool_b1 = tc.alloc_tile_pool(name="kv1", bufs=1)
    apool = tc.alloc_tile_pool(name="attn", bufs=4)
    y4pool = tc.alloc_tile_pool(name="y4", bufs=3)
    opool = tc.alloc_tile_pool(name="outev", bufs=6)
    ps2pool = tc.alloc_tile_pool(name="ps2", bufs=2, space="PSUM")
    ps_one = tc.alloc_tile_pool(name="ps_one", bufs=4, space="PSUM")

    # b1 k/v fetch right away (sync queue): streams during early b0 blocks
    kts_b1 = []
    vts_b1 = []
    for kc in range(T // 128):
        ktile = kvpool_b1.tile([128, 128], BF16, tag=f"k1t{kc}",
                               name=f"k1t{kc}")
        nc.sync.dma_start(ktile[:], kT_s[1][:, kc * 128:(kc + 1) * 128])
        kts_b1.append(ktile)
        vtile = kvpool_b1.tile([128, 128], BF16, tag=f"v1t{kc}",
                               name=f"v1t{kc}")
        nc.sync.dma_start(vtile[:], v_s[1][:, kc, :])
        vts_b1.append(vtile)

    # ---- phase 2: attention per (batch, head, query block) ----
    blocks = [(b, h, qb) for b in range(B) for h in range(QH)
              for qb in range(NQB)]

    def fetch_qT(i):
        b, h, qb = blocks[i]
        qT_sb = qpool.tile([128, QB], BF16, tag="qT", name="qT_sb")
        nc.scalar.dma_start(qT_sb[:], qT_s[b][h][:, qb * QB:(qb + 1) * QB])
        return qT_sb

    cur_qT = fetch_qT(0)
    for i, (b, h, qb) in enumerate(blocks):
        kts, vts = kv_pre if b == 0 else (kts_b1, vts_b1)
        nkc = 4 * (qb + 1)
        npair = nkc // 2
        qT_sb = cur_qT
        if i + 1 < len(blocks):
            cur_qT = fetch_qT(i + 1)

        ps_yt = ps_one.tile([128, 512], F32, tag="one", name="ps_yt")
        ps_bc = ps_one.tile([128, 512], F32, tag="one", name="ps_bc")
        a2s = []

        def consume(ip, nkc=nkc, npair=npair, ps_yt=ps_yt, ps_bc=ps_bc,
                    a2s=a2s, vts=vts):
            a2, asum = a2s[ip]
            for j in range(2):
                c = 2 * ip + j
                nc.tensor.matmul(ps_yt[:], vts[c][:], a2[:, j, :],
                                 start=(c == 0), stop=(c == nkc - 1))
            nc.tensor.matmul(ps_bc[:], onesPP[:], asum[:],
                             start=(ip == 0), stop=(ip == npair - 1))

        for ip in range(npair):
            ps2 = ps2pool.tile([128, 2, 512], F32, tag="s", name="ps2")
            for j in range(2):
                c = 2 * ip + j
                nc.tensor.matmul(ps2[:, j, :], kts[c][:], qT_sb[:],
                                 start=True, stop=True)
            a2 = apool.tile([128, 2, 512], BF16, tag="a", name="a2")
            nc.scalar.activation(a2[:], ps2[:], EXP, scale=SCALE)
            for j in range(2):
                c = 2 * ip + j
                delta = c * 128 - qb * QB
                if delta >= 0:
                    off = 384 - delta
                    nc.vector.tensor_mul(
                        out=a2[:, j, :], in0=a2[:, j, :],
                        in1=BIG[:, off:off + QB])
            asum = apool.tile([128, 512], BF16, tag="as", name="asum")
            nc.vector.tensor_add(out=asum[:], in0=a2[:, 0, :],
                                 in1=a2[:, 1, :])
            a2s.append((a2, asum))
            if ip >= 1:
                consume(ip - 1)
        consume(npair - 1)

        rb = qpool.tile([128, QB], F32, tag="rb", name="rb")
        nc.vector.reciprocal_approx_fast(out=rb[:], in_=ps_bc[:])
        yt = qpool.tile([128, QB], BF16, tag="yt", name="yt")
        nc.vector.tensor_mul(out=yt[:], in0=ps_yt[:], in1=rb[:])
        nc.sync.dma_start(yT_s[b][h][:, qb * QB:(qb + 1) * QB], yt[:])

    # ---- phase 3: out_partial = y @ wo_rows ----
    for b in range(B):
        for tg in range(T // 512):
            y4 = y4pool.tile([128, QH, 512], BF16, tag="y4", name="y4")
            for fc in range(QH):
                nc.sync.dma_start(
                    y4[:, fc, :], yT_s[b][fc][:, tg * 512:(tg + 1) * 512])
            for tcl in range(4):
                for oc in range(D // 512):
                    ps = ps_one.tile([128, 512], F32, tag="one", name="pso")
                    for fc in range(QH):
                        nc.tensor.matmul(
                            ps[:],
                            y4[:, fc, tcl * 128:(tcl + 1) * 128],
                            wo_sb[:, fc, oc * 512:(oc + 1) * 512],
                            start=(fc == 0), stop=(fc == QH - 1))
                    ot = opool.tile([128, 512], BF16, tag="ot", name="ot")
                    nc.any.tensor_copy(ot[:], ps[:])
                    row0 = b * T + tg * 512 + tcl * 128
                    nc.sync.dma_start(
                        out[row0:row0 + 128, oc * 512:(oc + 1) * 512], ot[:])

    opool.release()
    y4pool.release()
    apool.release()
    kvpool_b1.release()
    ps_one.release()
    ps2pool.release()
    qpool.release()
    kvpool_b0.release()
    wopool.release()
    const.release()
    dram.release()


_PERM = np.concatenate([np.arange(0, HD, 2), np.arange(1, HD, 2)])


def _prep_inputs(x, freqs_cis, wq, wk, wv, wo):
    x = np.asarray(x, dtype=np.float32)
    freqs_cis = np.asarray(freqs_cis, dtype=np.float32)
    wq = np.asarray(wq, dtype=np.float32)
    wk = np.asarray(wk, dtype=np.float32)
    wv = np.asarray(wv, dtype=np.float32)
    wo = np.asarray(wo, dtype=np.float32)

    x2 = x.reshape(BT, D)
    # [di, tchunk, dc, tlocal] so each phase-1 chunk DMA is 128 x 32KB contig
    xTq = np.ascontiguousarray(
        x2.reshape(NTC, TCH, DC, 128).transpose(3, 0, 2, 1)).astype(BF)

    cosv = freqs_cis[:, :, 0].T                      # [64, T]
    sinv = freqs_cis[:, :, 1].T
    cc1 = np.concatenate([cosv, cosv], axis=0)       # [128, T]
    ss1 = np.concatenate([-sinv, sinv], axis=0)
    cc = np.ascontiguousarray(np.tile(cc1, (1, B)))  # [128, B*T]
    ss = np.ascontiguousarray(np.tile(ss1, (1, B)))

    in_maps = []
    for c in range(N_CORES):
        qcols = np.concatenate(
            [(4 * c + h) * HD + _PERM for h in range(QH)])
        kcols = c * HD + _PERM
        in_maps.append({
            "xT": xTq,
            "wq": np.ascontiguousarray(
                wq[:, qcols].reshape(DC, 128, QF).transpose(1, 0, 2))
                .astype(BF),
            "wk": np.ascontiguousarray(
                wk[:, kcols].reshape(DC, 128, KF).transpose(1, 0, 2))
                .astype(BF),
            "wv": np.ascontiguousarray(
                wv[:, c * HD:(c + 1) * HD].reshape(DC, 128, KF)
                .transpose(1, 0, 2)).astype(BF),
            "wo": np.ascontiguousarray(
                wo[c * QF:(c + 1) * QF, :].reshape(QH, 128, D)
                .transpose(1, 0, 2)).astype(BF),
            "cc": cc,
            "ss": ss,
        })
    return in_maps


def kernel(x, freqs_cis, wq, wk, wv, wo):
    if not _NC_CACHE:
        _NC_CACHE.append(build())
    nc = _NC_CACHE[0]
    in_maps = _prep_inputs(x, freqs_cis, wq, wk, wv, wo)
    res = None
    err = None
    for _attempt in range(3):
        try:
            res = bass_utils.run_bass_kernel_spmd(
                nc, in_maps, core_ids=list(range(N_CORES)))
            break
        except Exception as e:  # transient NRT device wedge: retry
            err = e
            import time as _time
            _time.sleep(5)
    if res is None:
        raise err
    acc = res.results[0]["out"].astype(np.float32)
    for i in range(1, N_CORES):
        acc += res.results[i]["out"].astype(np.float32)
    return acc.reshape(B, T, D)


if __name__ == "__main__":
    rng = np.random.default_rng(0)
    s = 1.0 / np.sqrt(D)
    inputs = {
        "x": rng.standard_normal((B, T, D), dtype=np.float32),
        "freqs_cis": rng.standard_normal((T, HD // 2, 2), dtype=np.float32),
        "wq": rng.standard_normal((D, NH * HD), dtype=np.float32) * s,
        "wk": rng.standard_normal((D, NKV * HD), dtype=np.float32) * s,
        "wv": rng.standard_normal((D, NKV * HD), dtype=np.float32) * s,
        "wo": rng.standard_normal((D, D), dtype=np.float32) * s,
    }
    out = kernel(**inputs)
    print("out", out.shape, out.dtype, float(np.abs(out).mean()))


# revision 11
# speedup vs baseline: 1.1882x; 1.1882x over previous
"""Tensor-parallel GQA attention forward for Trainium2, 8 NeuronCores.

Problem: nn_Attention (B=2, T=2048, D=4096, 32 q heads, 8 kv heads, hd=128).

Sharding (tensor-parallel over heads):
  - core c owns q heads 4c..4c+3 (512 features) and kv head c (128 features)
  - wq/wk/wv column-sharded, wo row-sharded; x replicated (pre-transposed on
    host to x^T [D, B*T] so projections need no on-device transpose)
  - each core returns its partial y @ wo_rows contribution; the host sums the
    8 partials (the unshard step for row-sharded wo).

Matmuls run in bf16 with fp32 PSUM accumulation; softmax sums/reciprocal
stay fp32 (denominators via column-group-tiled ones-matmuls).

Device dataflow per core:
  P1: q^T/k^T/v^T = W^T x^T (PSUM accum over 32 d-chunks; weights preloaded
      with a few large DMAs ordered wk,wv,wq so the PE is never starved),
      RoPE fused on q^T/k^T via host-permuted even/odd feature order,
      v transposed to [token, d] tiles via bf16 PE transpose.
  P2: per (batch, head, 512-query block): scores^T = k^T.T @ q^T in pairs of
      key chunks -> one Exp per pair (ScalarE) -> causal band mask on the 4
      diagonal chunks -> y~^T = v.T @ attn^T (PSUM accum); denominators via a
      bf16 DVE pair-sum + one ones-matmul per pair -> y = y~^T * reciprocal.
  P3: out_partial = y^T.T @ wo_rows (PSUM accum over 4 feature chunks).
"""

import sys
import types

import numpy as np
import ml_dtypes

BF = ml_dtypes.bfloat16

B = 2
T = 2048
D = 4096
BT = B * T
NH = 32
NKV = 8
HD = 128
N_CORES = 8
QH = NH // N_CORES          # 4 q heads per core
QF = QH * HD                # 512 q features per core
KF = HD                     # 128 kv features per core
TCH = 256                   # phase-1 token chunk
NTC = BT // TCH             # 16 chunks
DC = D // 128               # 32 contraction chunks
QB = 512                    # phase-2 query block
NQB = T // QB               # 4 blocks per (batch, head)
SCALE = 1.0 / float(np.sqrt(HD))


def _install_ntff_hook_shim():
    """antenv.axon_hooks is absent in this image; synthesize it so
    run_bass_kernel_spmd(trace=True) can profile via libaxon_pjrt.so."""
    try:
        from antenv import axon_hooks  # noqa: F401
        return
    except ImportError:
        pass
    try:
        from trn_agent_boot.trn_boot import _ntff_profile_via_ctypes
        hook = _ntff_profile_via_ctypes("/opt/axon/libaxon_pjrt.so")
    except Exception:
        hook = None
    mod = types.ModuleType("antenv.axon_hooks")
    mod._hook = hook
    mod.get_axon_ntff_profile_hook = lambda: mod._hook

    def _set(h):
        mod._hook = h

    mod.set_axon_ntff_profile_hook = _set
    sys.modules["antenv.axon_hooks"] = mod


_install_ntff_hook_shim()

import concourse.bass as bass  # noqa: E402,F401
import concourse.bacc as bacc  # noqa: E402
import concourse.tile as tile  # noqa: E402
import concourse.mybir as mybir  # noqa: E402
from concourse import bass_utils  # noqa: E402
from concourse.masks import make_identity  # noqa: E402

F32 = mybir.dt.float32
F32R = mybir.dt.float32r
BF16 = mybir.dt.bfloat16
EXP = mybir.ActivationFunctionType.Exp

_NC_CACHE = []


def build():
    nc = bacc.Bacc("TRN2", target_bir_lowering=False, debug=False,
                   num_devices=N_CORES)

    xT = nc.dram_tensor("xT", [128, NTC, DC, TCH], BF16, kind="ExternalInput").ap()
    wq = nc.dram_tensor("wq", [128, DC, QF], BF16, kind="ExternalInput").ap()
    wk = nc.dram_tensor("wk", [128, DC, KF], BF16, kind="ExternalInput").ap()
    wv = nc.dram_tensor("wv", [128, DC, KF], BF16, kind="ExternalInput").ap()
    wo = nc.dram_tensor("wo", [128, QH, D], BF16, kind="ExternalInput").ap()
    cc = nc.dram_tensor("cc", [128, BT], F32, kind="ExternalInput").ap()
    ss = nc.dram_tensor("ss", [128, BT], F32, kind="ExternalInput").ap()
    out = nc.dram_tensor("out", [BT, D], BF16, kind="ExternalOutput").ap()

    with tile.TileContext(nc) as tc:
        _build_body(nc, tc, xT, wq, wk, wv, wo, cc, ss, out)
    nc.compile()
    return nc


def _build_body(nc, tc, xT, wq, wk, wv, wo, cc, ss, out):
    dram = tc.alloc_tile_pool(name="dram", bufs=1, space="DRAM")
    const = tc.alloc_tile_pool(name="const", bufs=1)
    # right-side pools live across the whole kernel (no aliasing with the
    # released phase-1 pools -> their DMAs never gate on phase-1 matmuls)
    wopool = tc.alloc_tile_pool(name="wo", bufs=1, side="right")
    kvpool_b0 = tc.alloc_tile_pool(name="kv0", bufs=1, side="right")
    qpool = tc.alloc_tile_pool(name="q", bufs=3, side="right")
    # PSUM: phase 1 uses ps_proj (4 banks) + ps_t (transposes); phase 2/3
    # use ps4 (2x2 banks) + ps_one ring (4 single banks).
    ps_proj = tc.alloc_tile_pool(name="ps_proj", bufs=4, space="PSUM")
    ps_t = tc.alloc_tile_pool(name="ps_t", bufs=2, space="PSUM")
    # phase-1 pools (released after phase 1 is emitted)
    wpool = tc.alloc_tile_pool(name="weights", bufs=1)
    xpool = tc.alloc_tile_pool(name="xstream", bufs=3)
    cspool = tc.alloc_tile_pool(name="cs", bufs=3)
    rpool = tc.alloc_tile_pool(name="rope", bufs=4)

    # ---- DRAM scratch (per-batch split for coarse phase overlap) ----
    qT_s = [dram.tile([QH, 128, T], BF16, tag=f"qTs{b}", name=f"qTs{b}") for b in range(B)]
    kT_s = [dram.tile([128, T], BF16, tag=f"kTs{b}", name=f"kTs{b}") for b in range(B)]
    v_s = [dram.tile([128, T // 128, 128], BF16, tag=f"vs{b}", name=f"vs{b}") for b in range(B)]
    yT_s = [dram.tile([QH, 128, T], BF16, tag=f"yTs{b}", name=f"yTs{b}") for b in range(B)]

    wo_sb = wopool.tile([128, QH, D], BF16)

    # ---- constants ----
    ident = const.tile([128, 128], F32)
    make_identity(nc, ident[:])
    ident_bf = const.tile([128, 128], BF16)
    nc.vector.tensor_copy(ident_bf[:], ident[:])
    onesPP = const.tile([128, 128], BF16)
    nc.vector.memset(onesPP[:], 1.0)
    # BIG[p, v] = 1.0 iff v - 384 >= p ; mask(delta) = BIG[:, 384-delta :][:QB]
    BIGf = const.tile([128, 896], F32)
    nc.gpsimd.memset(BIGf[:], 1.0)
    nc.gpsimd.affine_select(
        out=BIGf[:], in_=BIGf[:], compare_op=mybir.AluOpType.is_ge,
        fill=0.0, base=-384, channel_multiplier=-1, pattern=[[1, 896]],
    )
    BIG = const.tile([128, 896], BF16)
    nc.vector.tensor_copy(BIG[:], BIGf[:])

    # ---- phase 1: projections + RoPE ----
    # DMA order on the sync queue: x chunk 0, cos/sin 0, then weights as a
    # few large transfers (wk, wv first: the k/v projections of chunk 0 run
    # while wq streams). The x ring (bufs=3) gives a 2-chunk runway.
    def fetch_x(t, split=1):
        xts = []
        npc = DC // 4
        for sx in range(4):
            xst = xpool.tile([128, npc, TCH], BF16, tag=f"xt{sx}",
                             name=f"xt{sx}")
            for p in range(split):
                lo, hi = p * npc // split, (p + 1) * npc // split
                nc.sync.dma_start(
                    xst[:, lo:hi, :], xT[:, t, sx * npc + lo:sx * npc + hi, :])
            xts.append(xst)
        return xts

    def fetch_cs(t):
        tsl = slice(t * TCH, (t + 1) * TCH)
        cc_t = cspool.tile([128, TCH], F32, tag="cc")
        nc.scalar.dma_start(cc_t[:], cc[:, tsl])
        ss_t = cspool.tile([128, TCH], F32, tag="ss")
        nc.scalar.dma_start(ss_t[:], ss[:, tsl])
        return cc_t, ss_t

    cur_x = fetch_x(0, split=2)
    cur_cs = fetch_cs(0)

    wk_sb = wpool.tile([128, DC, KF], BF16, name="wk_sb")
    for piece in range(4):
        nc.scalar.dma_start(wk_sb[:, piece * 8:(piece + 1) * 8, :],
                            wk[:, piece * 8:(piece + 1) * 8, :])
    wv_sb = wpool.tile([128, DC, KF], BF16, name="wv_sb")
    for half in range(2):
        nc.scalar.dma_start(wv_sb[:, half * 16:(half + 1) * 16, :],
                            wv[:, half * 16:(half + 1) * 16, :])
    wq_sb = wpool.tile([128, DC, QF], BF16, name="wq_sb")
    for piece in range(8):
        nc.sync.dma_start(wq_sb[:, piece * 4:(piece + 1) * 4, :],
                          wq[:, piece * 4:(piece + 1) * 4, :])

    def rope_evict(ps, cc_t, ss_t, dst_ap):
        """psum [128, TCH] -> RoPE (fp32) -> bf16 -> DMA to dst_ap."""
        raw = rpool.tile([128, TCH], F32, tag="rraw")
        nc.any.tensor_copy(raw[:], ps[:])
        swp = rpool.tile([128, TCH], F32, tag="rswp")
        nc.vector.tensor_copy(swp[0:64, :], raw[64:128, :])
        nc.vector.tensor_copy(swp[64:128, :], raw[0:64, :])
        t1 = rpool.tile([128, TCH], BF16, tag="rt1")
        nc.vector.tensor_mul(out=swp[:], in0=swp[:], in1=ss_t[:])
        nc.vector.tensor_mul(out=t1[:], in0=raw[:], in1=cc_t[:])
        nc.vector.tensor_add(out=t1[:], in0=t1[:], in1=swp[:])
        nc.sync.dma_start(dst_ap, t1[:])

    kv_pre = ([], [])
    for t in range(NTC):
        b, tloc = divmod(t * TCH, T)
        lsl = slice(tloc, tloc + TCH)
        xts = cur_x
        cc_t, ss_t = cur_cs
        if t + 1 < NTC:
            cur_x = fetch_x(t + 1)
            cur_cs = fetch_cs(t + 1)

        ps = ps_proj.tile([128, 512], F32, tag="big", name="ps")[:, :TCH]
        for dc in range(DC):
            nc.tensor.matmul(ps[:], wk_sb[:, dc, :], xts[dc // 8][:, dc % 8, :],
                             start=(dc == 0), stop=(dc == DC - 1))
        rope_evict(ps, cc_t, ss_t, kT_s[b][:, lsl])

        ps = ps_proj.tile([128, 512], F32, tag="big", name="ps")[:, :TCH]
        for dc in range(DC):
            nc.tensor.matmul(ps[:], wv_sb[:, dc, :], xts[dc // 8][:, dc % 8, :],
                             start=(dc == 0), stop=(dc == DC - 1))
        vraw = rpool.tile([128, TCH], BF16, tag="vraw")
        nc.any.tensor_copy(vraw[:], ps[:])
        for j in range(TCH // 128):
            pst = ps_t.tile([128, 128], BF16, tag="pst", name="pst")
            nc.tensor.transpose(pst[:], vraw[:, j * 128:(j + 1) * 128],
                                ident_bf[:])
            vt = rpool.tile([128, 128], BF16, tag="vt")
            nc.any.tensor_copy(vt[:], pst[:])
            g = (tloc // 128) + j
            nc.sync.dma_start(v_s[b][:, g, :], vt[:])

        for fc in range(QH):
            ps = ps_proj.tile([128, 512], F32, tag="big", name="ps")[:, :TCH]
            for dc in range(DC):
                nc.tensor.matmul(
                    ps[:], wq_sb[:, dc, fc * 128:(fc + 1) * 128],
                    xts[dc // 8][:, dc % 8, :],
                    start=(dc == 0), stop=(dc == DC - 1))
            rope_evict(ps, cc_t, ss_t, qT_s[b][fc][:, lsl])

        if t == 2:
            # wo preload on the (otherwise idle) gpsimd SWDGE queue; delayed
            # past startup so it doesn't steal DMA bandwidth from x0/weights
            for fc in range(QH):
                nc.gpsimd.dma_start(wo_sb[:, fc, :], wo[:, fc, :])
        if t == 8:
            # b0 k/v prefetch on the scalar DMA queue (bypasses the x-stream
            # FIFO); data deps (chunk 0-7 scratch writes) already satisfied.
            for kc in range(T // 128):
                ktile = kvpool_b0.tile([128, 128], BF16, tag=f"k0t{kc}",
                                       name=f"k0t{kc}")
                nc.scalar.dma_start(ktile[:], kT_s[0][:, kc * 128:(kc + 1) * 128])
                kv_pre[0].append(ktile)
                vtile = kvpool_b0.tile([128, 128], BF16, tag=f"v0t{kc}",
                                       name=f"v0t{kc}")
                nc.scalar.dma_start(vtile[:], v_s[0][:, kc, :])
                kv_pre[1].append(vtile)

    rpool.release()
    cspool.release()
    xpool.release()
    wpool.release()
    ps_t.release()
    ps_proj.release()

    kvpool_b1 = tc.alloc_tile_pool(name="kv1", bufs=1)
    apool = tc.alloc_tile_pool(name="attn", bufs=4)
    y4pool = tc.alloc_tile_pool(name="y4", bufs=3)
    opool = tc.alloc_tile_pool(name="outev", bufs=6)
    ps2pool = tc.alloc_tile_pool(name="ps2", bufs=2, space="PSUM")
    ps_one = tc.alloc_tile_pool(name="ps_one", bufs=4, space="PSUM")

    # b1 k/v fetch right away (sync queue): streams during early b0 blocks
    kts_b1 = []
    vts_b1 = []
    for kc in range(T // 128):
        ktile = kvpool_b1.tile([128, 128], BF16, tag=f"k1t{kc}",
                               name=f"k1t{kc}")
        nc.sync.dma_start(ktile[:], kT_s[1][:, kc * 128:(kc + 1) * 128])
        kts_b1.append(ktile)
        vtile = kvpool_b1.tile([128, 128], BF16, tag=f"v1t{kc}",
                               name=f"v1t{kc}")
        nc.sync.dma_start(vtile[:], v_s[1][:, kc, :])
        vts_b1.append(vtile)

    # ---- phase 2: attention per (batch, head, query block) ----
    blocks = [(b, h, qb) for b in range(B) for h in range(QH)
              for qb in range(NQB)]

    def fetch_qT(i):
        b, h, qb = blocks[i]
        qT_sb = qpool.tile([128, QB], BF16, tag="qT", name="qT_sb")
        nc.scalar.dma_start(qT_sb[:], qT_s[b][h][:, qb * QB:(qb + 1) * QB])
        return qT_sb

    cur_qT = fetch_qT(0)
    for i, (b, h, qb) in enumerate(blocks):
        kts, vts = kv_pre if b == 0 else (kts_b1, vts_b1)
        nkc = 4 * (qb + 1)
        npair = nkc // 2
        qT_sb = cur_qT
        if i + 1 < len(blocks):
            cur_qT = fetch_qT(i + 1)

        ps_yt = ps_one.tile([128, 512], F32, tag="one", name="ps_yt")
        ps_bc = ps_one.tile([128, 512], F32, tag="one", name="ps_bc")
        a2s = []

        def consume(ip, nkc=nkc, npair=npair, ps_yt=ps_yt, ps_bc=ps_bc,
                    a2s=a2s, vts=vts):
            a2, asum = a2s[ip]
            for j in range(2):
                c = 2 * ip + j
                nc.tensor.matmul(ps_yt[:], vts[c][:], a2[:, j, :],
                                 start=(c == 0), stop=(c == nkc - 1))
            nc.tensor.matmul(ps_bc[:], onesPP[:], asum[:],
                             start=(ip == 0), stop=(ip == npair - 1))

        for ip in range(npair):
            ps2 = ps2pool.tile([128, 2, 512], F32, tag="s", name="ps2")
            for j in range(2):
                c = 2 * ip + j
                nc.tensor.matmul(ps2[:, j, :], kts[c][:], qT_sb[:],
                                 start=True, stop=True)
            a2 = apool.tile([128, 2, 512], BF16, tag="a", name="a2")
            nc.scalar.activation(a2[:], ps2[:], EXP, scale=SCALE)
            for j in range(2):
                c = 2 * ip + j
                delta = c * 128 - qb * QB
                if delta >= 0:
                    off = 384 - delta
                    nc.vector.tensor_mul(
                        out=a2[:, j, :], in0=a2[:, j, :],
                        in1=BIG[:, off:off + QB])
            asum = apool.tile([128, 512], BF16, tag="as", name="asum")
            nc.vector.tensor_add(out=asum[:], in0=a2[:, 0, :],
                                 in1=a2[:, 1, :])
            a2s.append((a2, asum))
            if ip >= 1:
                consume(ip - 1)
        consume(npair - 1)

        rb = qpool.tile([128, QB], F32, tag="rb", name="rb")
        nc.vector.reciprocal_approx_fast(out=rb[:], in_=ps_bc[:])
        yt = qpool.tile([128, QB], BF16, tag="yt", name="yt")
        nc.vector.tensor_mul(out=yt[:], in0=ps_yt[:], in1=rb[:])
        nc.sync.dma_start(yT_s[b][h][:, qb * QB:(qb + 1) * QB], yt[:])

    # ---- phase 3: out_partial = y @ wo_rows ----
    for b in range(B):
        for tg in range(T // 512):
            y4 = y4pool.tile([128, QH, 512], BF16, tag="y4", name="y4")
            for fc in range(QH):
                nc.sync.dma_start(
                    y4[:, fc, :], yT_s[b][fc][:, tg * 512:(tg + 1) * 512])
            for tcl in range(4):
                row0 = b * T + tg * 512 + tcl * 128
                for op2 in range(D // 1024):
                    ot = opool.tile([128, 2, 512], BF16, tag="ot", name="ot")
                    for half in range(2):
                        oc = op2 * 2 + half
                        ps = ps_one.tile([128, 512], F32, tag="one",
                                         name="pso")
                        for fc in range(QH):
                            nc.tensor.matmul(
                                ps[:],
                                y4[:, fc, tcl * 128:(tcl + 1) * 128],
                                wo_sb[:, fc, oc * 512:(oc + 1) * 512],
                                start=(fc == 0), stop=(fc == QH - 1))
                        nc.any.tensor_copy(ot[:, half, :], ps[:])
                    last = (b == B - 1 and tg == T // 512 - 1 and tcl == 3
                            and op2 == D // 1024 - 1)
                    nsplit = 4 if last else 1
                    for p in range(nsplit):
                        r0 = row0 + p * 128 // nsplit
                        r1 = row0 + (p + 1) * 128 // nsplit
                        nc.sync.dma_start(
                            out[r0:r1, op2 * 1024:(op2 + 1) * 1024],
                            ot[r0 - row0:r1 - row0, :, :])

    opool.release()
    y4pool.release()
    apool.release()
    kvpool_b1.release()
    ps_one.release()
    ps2pool.release()
    qpool.release()
    kvpool_b0.release()
    wopool.release()
    const.release()
    dram.release()


_PERM = np.concatenate([np.arange(0, HD, 2), np.arange(1, HD, 2)])


def _prep_inputs(x, freqs_cis, wq, wk, wv, wo):
    x = np.asarray(x, dtype=np.float32)
    freqs_cis = np.asarray(freqs_cis, dtype=np.float32)
    wq = np.asarray(wq, dtype=np.float32)
    wk = np.asarray(wk, dtype=np.float32)
    wv = np.asarray(wv, dtype=np.float32)
    wo = np.asarray(wo, dtype=np.float32)

    x2 = x.reshape(BT, D)
    # [di, tchunk, dc, tlocal] so each phase-1 chunk DMA is 128 x 32KB contig
    xTq = np.ascontiguousarray(
        x2.reshape(NTC, TCH, DC, 128).transpose(3, 0, 2, 1)).astype(BF)

    cosv = freqs_cis[:, :, 0].T                      # [64, T]
    sinv = freqs_cis[:, :, 1].T
    cc1 = np.concatenate([cosv, cosv], axis=0)       # [128, T]
    ss1 = np.concatenate([-sinv, sinv], axis=0)
    cc = np.ascontiguousarray(np.tile(cc1, (1, B)))  # [128, B*T]
    ss = np.ascontiguousarray(np.tile(ss1, (1, B)))

    in_maps = []
    for c in range(N_CORES):
        qcols = np.concatenate(
            [(4 * c + h) * HD + _PERM for h in range(QH)])
        kcols = c * HD + _PERM
        in_maps.append({
            "xT": xTq,
            "wq": np.ascontiguousarray(
                wq[:, qcols].reshape(DC, 128, QF).transpose(1, 0, 2))
                .astype(BF),
            "wk": np.ascontiguousarray(
                wk[:, kcols].reshape(DC, 128, KF).transpose(1, 0, 2))
                .astype(BF),
            "wv": np.ascontiguousarray(
                wv[:, c * HD:(c + 1) * HD].reshape(DC, 128, KF)
                .transpose(1, 0, 2)).astype(BF),
            "wo": np.ascontiguousarray(
                wo[c * QF:(c + 1) * QF, :].reshape(QH, 128, D)
                .transpose(1, 0, 2)).astype(BF),
            "cc": cc,
            "ss": ss,
        })
    return in_maps


def kernel(x, freqs_cis, wq, wk, wv, wo):
    if not _NC_CACHE:
        _NC_CACHE.append(build())
    nc = _NC_CACHE[0]
    in_maps = _prep_inputs(x, freqs_cis, wq, wk, wv, wo)
    res = None
    err = None
    for _attempt in range(3):
        try:
            res = bass_utils.run_bass_kernel_spmd(
                nc, in_maps, core_ids=list(range(N_CORES)))
            break
        except Exception as e:  # transient NRT device wedge: retry
            err = e
            import time as _time
            _time.sleep(5)
    if res is None:
        raise err
    acc = res.results[0]["out"].astype(np.float32)
    for i in range(1, N_CORES):
        acc += res.results[i]["out"].astype(np.float32)
    return acc.reshape(B, T, D)


if __name__ == "__main__":
    rng = np.random.default_rng(0)
    s = 1.0 / np.sqrt(D)
    inputs = {
        "x": rng.standard_normal((B, T, D), dtype=np.float32),
        "freqs_cis": rng.standard_normal((T, HD // 2, 2), dtype=np.float32),
        "wq": rng.standard_normal((D, NH * HD), dtype=np.float32) * s,
        "wk": rng.standard_normal((D, NKV * HD), dtype=np.float32) * s,
        "wv": rng.standard_normal((D, NKV * HD), dtype=np.float32) * s,
        "wo": rng.standard_normal((D, D), dtype=np.float32) * s,
    }
    out = kernel(**inputs)
    print("out", out.shape, out.dtype, float(np.abs(out).mean()))


# revision 12
# speedup vs baseline: 1.2106x; 1.0188x over previous
"""Tensor-parallel GQA attention forward for Trainium2, 8 NeuronCores.

Problem: nn_Attention (B=2, T=2048, D=4096, 32 q heads, 8 kv heads, hd=128).

Sharding (tensor-parallel over heads):
  - core c owns q heads 4c..4c+3 (512 features) and kv head c (128 features)
  - wq/wk/wv column-sharded, wo row-sharded; x replicated (pre-transposed on
    host to x^T [D, B*T] so projections need no on-device transpose)
  - each core returns its partial y @ wo_rows contribution; the host sums the
    8 partials (the unshard step for row-sharded wo).

Matmuls run in bf16 with fp32 PSUM accumulation; softmax sums/reciprocal
stay fp32 (denominators via column-group-tiled ones-matmuls).

Device dataflow per core:
  P1: q^T/k^T/v^T = W^T x^T (PSUM accum over 32 d-chunks; weights preloaded
      with a few large DMAs ordered wk,wv,wq so the PE is never starved),
      RoPE fused on q^T/k^T via host-permuted even/odd feature order,
      v transposed to [token, d] tiles via bf16 PE transpose.
  P2: per (batch, head, 512-query block): scores^T = k^T.T @ q^T in pairs of
      key chunks -> one Exp per pair (ScalarE) -> causal band mask on the 4
      diagonal chunks -> y~^T = v.T @ attn^T (PSUM accum); denominators via a
      bf16 DVE pair-sum + one ones-matmul per pair -> y = y~^T * reciprocal.
  P3: out_partial = y^T.T @ wo_rows (PSUM accum over 4 feature chunks).
"""

import sys
import types

import numpy as np
import ml_dtypes

BF = ml_dtypes.bfloat16

B = 2
T = 2048
D = 4096
BT = B * T
NH = 32
NKV = 8
HD = 128
N_CORES = 8
QH = NH // N_CORES          # 4 q heads per core
QF = QH * HD                # 512 q features per core
KF = HD                     # 128 kv features per core
TCH = 256                   # phase-1 token chunk
NTC = BT // TCH             # 16 chunks
DC = D // 128               # 32 contraction chunks
QB = 512                    # phase-2 query block
NQB = T // QB               # 4 blocks per (batch, head)
SCALE = 1.0 / float(np.sqrt(HD))


def _install_ntff_hook_shim():
    """antenv.axon_hooks is absent in this image; synthesize it so
    run_bass_kernel_spmd(trace=True) can profile via libaxon_pjrt.so."""
    try:
        from antenv import axon_hooks  # noqa: F401
        return
    except ImportError:
        pass
    try:
        from trn_agent_boot.trn_boot import _ntff_profile_via_ctypes
        hook = _ntff_profile_via_ctypes("/opt/axon/libaxon_pjrt.so")
    except Exception:
        hook = None
    mod = types.ModuleType("antenv.axon_hooks")
    mod._hook = hook
    mod.get_axon_ntff_profile_hook = lambda: mod._hook

    def _set(h):
        mod._hook = h

    mod.set_axon_ntff_profile_hook = _set
    sys.modules["antenv.axon_hooks"] = mod


_install_ntff_hook_shim()

import concourse.bass as bass  # noqa: E402,F401
import concourse.bacc as bacc  # noqa: E402
import concourse.tile as tile  # noqa: E402
import concourse.mybir as mybir  # noqa: E402
from concourse import bass_utils  # noqa: E402
from concourse.masks import make_identity  # noqa: E402

F32 = mybir.dt.float32
F32R = mybir.dt.float32r
BF16 = mybir.dt.bfloat16
EXP = mybir.ActivationFunctionType.Exp

_NC_CACHE = []


def build():
    nc = bacc.Bacc("TRN2", target_bir_lowering=False, debug=False,
                   num_devices=N_CORES)

    xT = nc.dram_tensor("xT", [128, NTC, DC, TCH], BF16, kind="ExternalInput").ap()
    wq = nc.dram_tensor("wq", [128, DC, QF], BF16, kind="ExternalInput").ap()
    wk = nc.dram_tensor("wk", [128, DC, KF], BF16, kind="ExternalInput").ap()
    wv = nc.dram_tensor("wv", [128, DC, KF], BF16, kind="ExternalInput").ap()
    wo = nc.dram_tensor("wo", [128, QH, D], BF16, kind="ExternalInput").ap()
    cc = nc.dram_tensor("cc", [128, BT], F32, kind="ExternalInput").ap()
    ss = nc.dram_tensor("ss", [128, BT], F32, kind="ExternalInput").ap()
    out = nc.dram_tensor("out", [BT, D], BF16, kind="ExternalOutput").ap()

    with tile.TileContext(nc) as tc:
        _build_body(nc, tc, xT, wq, wk, wv, wo, cc, ss, out)
    nc.compile()
    return nc


def _build_body(nc, tc, xT, wq, wk, wv, wo, cc, ss, out):
    dram = tc.alloc_tile_pool(name="dram", bufs=1, space="DRAM")
    const = tc.alloc_tile_pool(name="const", bufs=1)
    # right-side pools live across the whole kernel (no aliasing with the
    # released phase-1 pools -> their DMAs never gate on phase-1 matmuls)
    wopool = tc.alloc_tile_pool(name="wo", bufs=1, side="right")
    kvpool_b0 = tc.alloc_tile_pool(name="kv0", bufs=1, side="right")
    qpool = tc.alloc_tile_pool(name="q", bufs=3, side="right")
    # PSUM: phase 1 uses ps_proj (4 banks) + ps_t (transposes); phase 2/3
    # use ps4 (2x2 banks) + ps_one ring (4 single banks).
    ps_proj = tc.alloc_tile_pool(name="ps_proj", bufs=4, space="PSUM")
    ps_t = tc.alloc_tile_pool(name="ps_t", bufs=2, space="PSUM")
    # phase-1 pools (released after phase 1 is emitted)
    wpool = tc.alloc_tile_pool(name="weights", bufs=1)
    xpool = tc.alloc_tile_pool(name="xstream", bufs=3)
    cspool = tc.alloc_tile_pool(name="cs", bufs=3)
    rpool = tc.alloc_tile_pool(name="rope", bufs=4)

    # ---- DRAM scratch (per-batch split for coarse phase overlap) ----
    qT_s = [dram.tile([QH, 128, T], BF16, tag=f"qTs{b}", name=f"qTs{b}") for b in range(B)]
    kT_s = [dram.tile([128, T], BF16, tag=f"kTs{b}", name=f"kTs{b}") for b in range(B)]
    v_s = [dram.tile([128, T // 128, 128], BF16, tag=f"vs{b}", name=f"vs{b}") for b in range(B)]
    yT_s = [dram.tile([QH, 128, T], BF16, tag=f"yTs{b}", name=f"yTs{b}") for b in range(B)]

    wo_sb = wopool.tile([128, QH, D], BF16)

    # ---- constants ----
    ident = const.tile([128, 128], F32)
    make_identity(nc, ident[:])
    ident_bf = const.tile([128, 128], BF16)
    nc.vector.tensor_copy(ident_bf[:], ident[:])
    onesPP = const.tile([128, 128], BF16)
    nc.vector.memset(onesPP[:], 1.0)
    # BIG[p, v] = 1.0 iff v - 384 >= p ; mask(delta) = BIG[:, 384-delta :][:QB]
    BIGf = const.tile([128, 896], F32)
    nc.gpsimd.memset(BIGf[:], 1.0)
    nc.gpsimd.affine_select(
        out=BIGf[:], in_=BIGf[:], compare_op=mybir.AluOpType.is_ge,
        fill=0.0, base=-384, channel_multiplier=-1, pattern=[[1, 896]],
    )
    BIG = const.tile([128, 896], BF16)
    nc.vector.tensor_copy(BIG[:], BIGf[:])

    # ---- phase 1: projections + RoPE ----
    # DMA order on the sync queue: x chunk 0, cos/sin 0, then weights as a
    # few large transfers (wk, wv first: the k/v projections of chunk 0 run
    # while wq streams). The x ring (bufs=3) gives a 2-chunk runway.
    def fetch_x(t, split=1):
        xts = []
        npc = DC // 4
        for sx in range(4):
            xst = xpool.tile([128, npc, TCH], BF16, tag=f"xt{sx}",
                             name=f"xt{sx}")
            for p in range(split):
                lo, hi = p * npc // split, (p + 1) * npc // split
                nc.sync.dma_start(
                    xst[:, lo:hi, :], xT[:, t, sx * npc + lo:sx * npc + hi, :])
            xts.append(xst)
        return xts

    def fetch_cs(t):
        tsl = slice(t * TCH, (t + 1) * TCH)
        cc_t = cspool.tile([128, TCH], F32, tag="cc")
        nc.scalar.dma_start(cc_t[:], cc[:, tsl])
        ss_t = cspool.tile([128, TCH], F32, tag="ss")
        nc.scalar.dma_start(ss_t[:], ss[:, tsl])
        return cc_t, ss_t

    cur_x = fetch_x(0, split=2)
    cur_cs = fetch_cs(0)

    wk_sb = wpool.tile([128, DC, KF], BF16, name="wk_sb")
    for piece in range(4):
        nc.scalar.dma_start(wk_sb[:, piece * 8:(piece + 1) * 8, :],
                            wk[:, piece * 8:(piece + 1) * 8, :])
    wv_sb = wpool.tile([128, DC, KF], BF16, name="wv_sb")
    for half in range(2):
        nc.scalar.dma_start(wv_sb[:, half * 16:(half + 1) * 16, :],
                            wv[:, half * 16:(half + 1) * 16, :])
    wq_sb = wpool.tile([128, DC, QF], BF16, name="wq_sb")
    for piece in range(8):
        nc.gpsimd.dma_start(wq_sb[:, piece * 4:(piece + 1) * 4, :],
                            wq[:, piece * 4:(piece + 1) * 4, :])

    def rope_evict(ps, cc_t, ss_t, dst_ap):
        """psum [128, TCH] -> RoPE (fp32) -> bf16 -> DMA to dst_ap."""
        raw = rpool.tile([128, TCH], F32, tag="rraw")
        nc.any.tensor_copy(raw[:], ps[:])
        swp = rpool.tile([128, TCH], F32, tag="rswp")
        nc.vector.tensor_copy(swp[0:64, :], raw[64:128, :])
        nc.vector.tensor_copy(swp[64:128, :], raw[0:64, :])
        t1 = rpool.tile([128, TCH], BF16, tag="rt1")
        nc.vector.tensor_mul(out=swp[:], in0=swp[:], in1=ss_t[:])
        nc.vector.tensor_mul(out=t1[:], in0=raw[:], in1=cc_t[:])
        nc.vector.tensor_add(out=t1[:], in0=t1[:], in1=swp[:])
        nc.sync.dma_start(dst_ap, t1[:])

    kv_pre = ([], [])
    for t in range(NTC):
        b, tloc = divmod(t * TCH, T)
        lsl = slice(tloc, tloc + TCH)
        xts = cur_x
        cc_t, ss_t = cur_cs
        if t + 1 < NTC:
            cur_x = fetch_x(t + 1)
            cur_cs = fetch_cs(t + 1)

        ps = ps_proj.tile([128, 512], F32, tag="big", name="ps")[:, :TCH]
        for dc in range(DC):
            nc.tensor.matmul(ps[:], wk_sb[:, dc, :], xts[dc // 8][:, dc % 8, :],
                             start=(dc == 0), stop=(dc == DC - 1))
        rope_evict(ps, cc_t, ss_t, kT_s[b][:, lsl])

        ps = ps_proj.tile([128, 512], F32, tag="big", name="ps")[:, :TCH]
        for dc in range(DC):
            nc.tensor.matmul(ps[:], wv_sb[:, dc, :], xts[dc // 8][:, dc % 8, :],
                             start=(dc == 0), stop=(dc == DC - 1))
        vraw = rpool.tile([128, TCH], BF16, tag="vraw")
        nc.any.tensor_copy(vraw[:], ps[:])
        for j in range(TCH // 128):
            pst = ps_t.tile([128, 128], BF16, tag="pst", name="pst")
            nc.tensor.transpose(pst[:], vraw[:, j * 128:(j + 1) * 128],
                                ident_bf[:])
            vt = rpool.tile([128, 128], BF16, tag="vt")
            nc.any.tensor_copy(vt[:], pst[:])
            g = (tloc // 128) + j
            nc.sync.dma_start(v_s[b][:, g, :], vt[:])

        for fc in range(QH):
            ps = ps_proj.tile([128, 512], F32, tag="big", name="ps")[:, :TCH]
            for dc in range(DC):
                nc.tensor.matmul(
                    ps[:], wq_sb[:, dc, fc * 128:(fc + 1) * 128],
                    xts[dc // 8][:, dc % 8, :],
                    start=(dc == 0), stop=(dc == DC - 1))
            rope_evict(ps, cc_t, ss_t, qT_s[b][fc][:, lsl])

        if t == 2:
            # wo preload on the (otherwise idle) gpsimd SWDGE queue; delayed
            # past startup so it doesn't steal DMA bandwidth from x0/weights
            for fc in range(QH):
                nc.gpsimd.dma_start(wo_sb[:, fc, :], wo[:, fc, :])
        if t == 8:
            # b0 k/v prefetch on the scalar DMA queue (bypasses the x-stream
            # FIFO); data deps (chunk 0-7 scratch writes) already satisfied.
            for kc in range(T // 128):
                ktile = kvpool_b0.tile([128, 128], BF16, tag=f"k0t{kc}",
                                       name=f"k0t{kc}")
                nc.scalar.dma_start(ktile[:], kT_s[0][:, kc * 128:(kc + 1) * 128])
                kv_pre[0].append(ktile)
                vtile = kvpool_b0.tile([128, 128], BF16, tag=f"v0t{kc}",
                                       name=f"v0t{kc}")
                nc.scalar.dma_start(vtile[:], v_s[0][:, kc, :])
                kv_pre[1].append(vtile)

    rpool.release()
    cspool.release()
    xpool.release()
    wpool.release()
    ps_t.release()
    ps_proj.release()

    kvpool_b1 = tc.alloc_tile_pool(name="kv1", bufs=1)
    apool = tc.alloc_tile_pool(name="attn", bufs=4)
    y4pool = tc.alloc_tile_pool(name="y4", bufs=3)
    opool = tc.alloc_tile_pool(name="outev", bufs=6)
    ps2pool = tc.alloc_tile_pool(name="ps2", bufs=2, space="PSUM")
    ps_one = tc.alloc_tile_pool(name="ps_one", bufs=4, space="PSUM")

    # b1 k/v fetch right away (sync queue): streams during early b0 blocks
    kts_b1 = []
    vts_b1 = []
    for kc in range(T // 128):
        ktile = kvpool_b1.tile([128, 128], BF16, tag=f"k1t{kc}",
                               name=f"k1t{kc}")
        nc.sync.dma_start(ktile[:], kT_s[1][:, kc * 128:(kc + 1) * 128])
        kts_b1.append(ktile)
        vtile = kvpool_b1.tile([128, 128], BF16, tag=f"v1t{kc}",
                               name=f"v1t{kc}")
        nc.sync.dma_start(vtile[:], v_s[1][:, kc, :])
        vts_b1.append(vtile)

    # ---- phase 2: attention per (batch, head, query block) ----
    blocks = [(b, h, qb) for b in range(B) for h in range(QH)
              for qb in range(NQB)]

    def fetch_qT(i):
        b, h, qb = blocks[i]
        qT_sb = qpool.tile([128, QB], BF16, tag="qT", name="qT_sb")
        nc.scalar.dma_start(qT_sb[:], qT_s[b][h][:, qb * QB:(qb + 1) * QB])
        return qT_sb

    cur_qT = fetch_qT(0)
    pending = []   # deferred consume thunks (global lag-1 across blocks)

    def drain_pending():
        while pending:
            pending.pop(0)()

    for i, (b, h, qb) in enumerate(blocks):
        kts, vts = kv_pre if b == 0 else (kts_b1, vts_b1)
        nkc = 4 * (qb + 1)
        npair = nkc // 2
        qT_sb = cur_qT
        if i + 1 < len(blocks):
            cur_qT = fetch_qT(i + 1)

        ps_yt = ps_one.tile([128, 512], F32, tag="one", name="ps_yt")
        ps_bc = ps_one.tile([128, 512], F32, tag="one", name="ps_bc")
        a2s = []
        finals = []

        def consume(ip, nkc=nkc, npair=npair, ps_yt=ps_yt, ps_bc=ps_bc,
                    a2s=a2s, vts=vts):
            a2, asum = a2s[ip]
            for j in range(2):
                c = 2 * ip + j
                nc.tensor.matmul(ps_yt[:], vts[c][:], a2[:, j, :],
                                 start=(c == 0), stop=(c == nkc - 1))
            nc.tensor.matmul(ps_bc[:], onesPP[:], asum[:],
                             start=(ip == 0), stop=(ip == npair - 1))

        def block_tail(b=b, h=h, qb=qb, ps_yt=ps_yt, ps_bc=ps_bc):
            rb = qpool.tile([128, QB], F32, tag="rb", name="rb")
            nc.vector.reciprocal_approx_fast(out=rb[:], in_=ps_bc[:])
            yt = qpool.tile([128, QB], BF16, tag="yt", name="yt")
            nc.vector.tensor_mul(out=yt[:], in0=ps_yt[:], in1=rb[:])
            nc.sync.dma_start(yT_s[b][h][:, qb * QB:(qb + 1) * QB], yt[:])

        for ip in range(npair):
            ps2 = ps2pool.tile([128, 2, 512], F32, tag="s", name="ps2")
            for j in range(2):
                c = 2 * ip + j
                nc.tensor.matmul(ps2[:, j, :], kts[c][:], qT_sb[:],
                                 start=True, stop=True)
            if ip == 0:
                # previous block's deferred tail runs behind our first
                # score pair, so its exp wait never idles the PE
                drain_pending()
            a2 = apool.tile([128, 2, 512], BF16, tag="a", name="a2")
            nc.scalar.activation(a2[:], ps2[:], EXP, scale=SCALE)
            for j in range(2):
                c = 2 * ip + j
                delta = c * 128 - qb * QB
                if delta >= 0:
                    off = 384 - delta
                    nc.vector.tensor_mul(
                        out=a2[:, j, :], in0=a2[:, j, :],
                        in1=BIG[:, off:off + QB])
            asum = apool.tile([128, 512], BF16, tag="as", name="asum")
            nc.vector.tensor_add(out=asum[:], in0=a2[:, 0, :],
                                 in1=a2[:, 1, :])
            a2s.append((a2, asum))
            if ip >= 2:
                consume(ip - 2)
        if npair >= 2:
            pending.append(lambda c=consume, n=npair: c(n - 2))
        pending.append(lambda c=consume, n=npair: c(n - 1))
        pending.append(block_tail)
    drain_pending()

    # ---- phase 3: out_partial = y @ wo_rows ----
    for b in range(B):
        for tg in range(T // 512):
            y4 = y4pool.tile([128, QH, 512], BF16, tag="y4", name="y4")
            for fc in range(QH):
                nc.sync.dma_start(
                    y4[:, fc, :], yT_s[b][fc][:, tg * 512:(tg + 1) * 512])
            for tcl in range(4):
                row0 = b * T + tg * 512 + tcl * 128
                for op2 in range(D // 1024):
                    ot = opool.tile([128, 2, 512], BF16, tag="ot", name="ot")
                    for half in range(2):
                        oc = op2 * 2 + half
                        ps = ps_one.tile([128, 512], F32, tag="one",
                                         name="pso")
                        for fc in range(QH):
                            nc.tensor.matmul(
                                ps[:],
                                y4[:, fc, tcl * 128:(tcl + 1) * 128],
                                wo_sb[:, fc, oc * 512:(oc + 1) * 512],
                                start=(fc == 0), stop=(fc == QH - 1))
                        nc.any.tensor_copy(ot[:, half, :], ps[:])
                    last = (b == B - 1 and tg == T // 512 - 1 and tcl == 3
                            and op2 == D // 1024 - 1)
                    nsplit = 4 if last else 1
                    for p in range(nsplit):
                        r0 = row0 + p * 128 // nsplit
                        r1 = row0 + (p + 1) * 128 // nsplit
                        nc.sync.dma_start(
                            out[r0:r1, op2 * 1024:(op2 + 1) * 1024],
                            ot[r0 - row0:r1 - row0, :, :])

    opool.release()
    y4pool.release()
    apool.release()
    kvpool_b1.release()
    ps_one.release()
    ps2pool.release()
    qpool.release()
    kvpool_b0.release()
    wopool.release()
    const.release()
    dram.release()


_PERM = np.concatenate([np.arange(0, HD, 2), np.arange(1, HD, 2)])


def _prep_inputs(x, freqs_cis, wq, wk, wv, wo):
    x = np.asarray(x, dtype=np.float32)
    freqs_cis = np.asarray(freqs_cis, dtype=np.float32)
    wq = np.asarray(wq, dtype=np.float32)
    wk = np.asarray(wk, dtype=np.float32)
    wv = np.asarray(wv, dtype=np.float32)
    wo = np.asarray(wo, dtype=np.float32)

    x2 = x.reshape(BT, D)
    # [di, tchunk, dc, tlocal] so each phase-1 chunk DMA is 128 x 32KB contig
    xTq = np.ascontiguousarray(
        x2.reshape(NTC, TCH, DC, 128).transpose(3, 0, 2, 1)).astype(BF)

    cosv = freqs_cis[:, :, 0].T                      # [64, T]
    sinv = freqs_cis[:, :, 1].T
    cc1 = np.concatenate([cosv, cosv], axis=0)       # [128, T]
    ss1 = np.concatenate([-sinv, sinv], axis=0)
    cc = np.ascontiguousarray(np.tile(cc1, (1, B)))  # [128, B*T]
    ss = np.ascontiguousarray(np.tile(ss1, (1, B)))

    in_maps = []
    for c in range(N_CORES):
        qcols = np.concatenate(
            [(4 * c + h) * HD + _PERM for h in range(QH)])
        kcols = c * HD + _PERM
        in_maps.append({
            "xT": xTq,
            "wq": np.ascontiguousarray(
                wq[:, qcols].reshape(DC, 128, QF).transpose(1, 0, 2))
                .astype(BF),
            "wk": np.ascontiguousarray(
                wk[:, kcols].reshape(DC, 128, KF).transpose(1, 0, 2))
                .astype(BF),
            "wv": np.ascontiguousarray(
                wv[:, c * HD:(c + 1) * HD].reshape(DC, 128, KF)
                .transpose(1, 0, 2)).astype(BF),
            "wo": np.ascontiguousarray(
                wo[c * QF:(c + 1) * QF, :].reshape(QH, 128, D)
                .transpose(1, 0, 2)).astype(BF),
            "cc": cc,
            "ss": ss,
        })
    return in_maps


def kernel(x, freqs_cis, wq, wk, wv, wo):
    if not _NC_CACHE:
        _NC_CACHE.append(build())
    nc = _NC_CACHE[0]
    in_maps = _prep_inputs(x, freqs_cis, wq, wk, wv, wo)
    res = None
    err = None
    for _attempt in range(3):
        try:
            res = bass_utils.run_bass_kernel_spmd(
                nc, in_maps, core_ids=list(range(N_CORES)))
            break
        except Exception as e:  # transient NRT device wedge: retry
            err = e
            import time as _time
            _time.sleep(5)
    if res is None:
        raise err
    acc = res.results[0]["out"].astype(np.float32)
    for i in range(1, N_CORES):
        acc += res.results[i]["out"].astype(np.float32)
    return acc.reshape(B, T, D)


if __name__ == "__main__":
    rng = np.random.default_rng(0)
    s = 1.0 / np.sqrt(D)
    inputs = {
        "x": rng.standard_normal((B, T, D), dtype=np.float32),
        "freqs_cis": rng.standard_normal((T, HD // 2, 2), dtype=np.float32),
        "wq": rng.standard_normal((D, NH * HD), dtype=np.float32) * s,
        "wk": rng.standard_normal((D, NKV * HD), dtype=np.float32) * s,
        "wv": rng.standard_normal((D, NKV * HD), dtype=np.float32) * s,
        "wo": rng.standard_normal((D, D), dtype=np.float32) * s,
    }
    out = kernel(**inputs)
    print("out", out.shape, out.dtype, float(np.abs(out).mean()))
